# revision 20
# baseline (speedup 1.0000x reference)
"""Trainium2 Bass kernel for nn_AttnResModule (pooling / memory-bound).

Computation (reference):
    inv_rms = rsqrt(mean(V*V, -1) + eps)
    logits  = einsum('d,nbtd->nbt', query, V) * inv_rms
    w       = softmax(logits, axis=0)            # over stack axis n=4
    out     = einsum('nbt,nbtd->btd', w, V)

Design (one row per partition; tile = [128 rows, (n=4, d=2048)] bf16):
  - stats: ONE fused custom-DVE scan op per (n, tile) computes BOTH
    q.V and sum(V^2) in a single 1-elem/cycle pass, interleaved by a
    parity scan:  out[k] = k odd ? cumsum(V*q)[k] : cumsum(V*V)[k].
    The dst AP is a stride-0 broadcast, so all 2048 writes land on 2
    columns and the final values are (ssq at col 0, dot at col 1).
    64 ops x ~2.28us = 146us on DVE -- the kernel's critical engine.
    (Alternatives measured on HW: STT+accum 2287ns *per stat*, ACT
    Square+accum 2320ns; tensor_reduce has no perf mode, 2750ns.)
  - inv_rms = exp(-0.5*ln(ssq/D + eps)) on ACT; /D+eps folded into
    Ln's scale/bias. The ACT table set (natural_log_exp_and_others:
    Ln+Exp+Copy) is pinned with an explicit InstLoadActFuncSet, which
    kills the inserter's 29-reload thrash (37us in an earlier rev).
  - softmax: Exp on ACT; sum over n is an in-partition tiny reduce;
    the 1/sum normalization folds into the PSUM->SBUF copies' scale,
    so combine weights are just diag(e_n).
  - combine: PE matmuls with diagonal lhsT built by ACT Copy with a
    per-partition AP scale; PSUM accumulates over n; chunk-major order
    so copies start early.
  - output: PSUM f32 -> SBUF bf16 copy-with-scale on ACT (vector for
    the last two tiles, ACT queue latency gates the tail), split
    stores via gpsimd SWDGE; host upcasts to f32 (store traffic halved)
  - pipeline: pair-delayed; the last two tiles run tile-granular so the
    second-to-last combine overlaps the last stats. v_bufs=6 is a
    measured optimum: deeper DMA look-ahead slows every engine via
    SBUF port contention (custom op 2284 -> 2740ns at v_bufs=10).

Sharding: data-parallel over rows; 8 cores x 2048 rows, no
communication. Measured: 257.7us (v1 baseline) -> ~180us, rel err 3.1e-3.
"""

import sys
from contextlib import ExitStack

import numpy as np

_TRN_REPO = "/opt/trn_rl_repo"
if _TRN_REPO not in sys.path:
    sys.path.insert(0, _TRN_REPO)

import concourse.bacc as bacc
import concourse.tile as tile
from concourse import mybir
from concourse.bass_utils import run_bass_kernel_spmd

N_STACK = 4
B = 4
T = 4096
D = 2048
N_CORES = 8
ROWS = B * T
ROWS_PER_CORE = ROWS // N_CORES
EPS = float(np.finfo(np.float32).eps)
F32 = mybir.dt.float32
BF16 = mybir.dt.bfloat16
FP16 = mybir.dt.float16

_STATS_OP_NAME = "ATTN_STATS_ANT"
STATS_PERF_MAX = 1  # byte-36[7:6]: highest engine-reachable perf slot (2x_1p)


def _build_stats_uops_2x():
    """Hand-authored 2x_1p datapath program for the fused dot/ssq op.

    Per cycle the engine delivers a packed pair: e0=SRC_0, e1=SRC_0_HI,
    q0=SRC_1, q1=SRC_1_HI, and writes the packed pair WR0_LO/WR0_HI.

        blk0: s0 = e0*e0          blk4: t = m0+m1
        blk1: m0 = e0*q0          blk5: u = s0+s1
        blk2: m1 = e1*q1          blk6: D += t   (CURR feedback scan)
        blk3: s1 = e1*e1          blk7: S += u   (CURR feedback scan)
        out:  WR0_LO = S (ssq cum incl. both), WR0_HI = D (dot cum)

    Mid-stream even slots differ from the 1x parity-select program (they
    include the pair's odd square), but consumers only read the last
    pair, where S/D are the full-stream ssq/dot.
    """
    from concourse.dve_uop import (
        AluInp, AluOp, DelayInp, InpSel, OutPath, OutSel, Trigger, UopConfig,
    )

    PD = [AluInp.PREV_DELAY_0, AluInp.PREV_DELAY_1, AluInp.PREV_DELAY_2,
          AluInp.PREV_DELAY_3, AluInp.PREV_DELAY_4]

    def mk_steady():
        u = UopConfig()
        for lane, src in ((1, InpSel.SRC_0), (2, InpSel.SRC_1),
                          (3, InpSel.SRC_0_HI), (4, InpSel.SRC_1_HI),
                          (5, InpSel.ZERO)):
            u.enable_input(src, lane)
        u.require_inp0 = 1
        u.require_inp1 = 1
        u.trigger = (Trigger.SRC_TENSOR_DONE, Trigger.NONE, Trigger.NONE)
        u.next_uop = (0, 0, 0)
        b = u.datapath_config
        # blk0: s0 = e0*e0; carry e0,q0,e1,q1
        b[0].enable_alu(AluOp.MULTIPLY, PD[0], PD[0])
        b[0].pass_through_delay(0, 1, 2, 3)
        # blk1: m0 = e0*q0; c0 <- s0, carry e1,q1
        b[1].enable_alu(AluOp.MULTIPLY, PD[0], PD[1])
        b[1].enable_delay_from_src(DelayInp.PREV_ALU_OUT, 0)
        b[1].pass_through_delay(2, 3)
        # blk2: m1 = e1*q1; c1 <- m0, carry s0,e1
        b[2].enable_alu(AluOp.MULTIPLY, PD[2], PD[3])
        b[2].enable_delay_from_src(DelayInp.PREV_ALU_OUT, 1)
        b[2].pass_through_delay(0, 2)
        # blk3: s1 = e1*e1; c3 <- m1, carry s0,m0
        b[3].enable_alu(AluOp.MULTIPLY, PD[2], PD[2])
        b[3].enable_delay_from_src(DelayInp.PREV_ALU_OUT, 3)
        b[3].pass_through_delay(0, 1)
        # blk4: t = m0+m1; c2 <- s1, carry s0
        b[4].enable_alu(AluOp.ADD, PD[1], PD[3])
        b[4].enable_delay_from_src(DelayInp.PREV_ALU_OUT, 2)
        b[4].pass_through_delay(0)
        # blk5: u = s0+s1; c1 <- t
        b[5].enable_alu(AluOp.ADD, PD[0], PD[2])
        b[5].enable_delay_from_src(DelayInp.PREV_ALU_OUT, 1)
        # blk6: D += t (scan); c0 <- u
        b[6].enable_alu(AluOp.ADD, AluInp.CURR_ALU_OUT, PD[1])
        b[6].enable_delay_from_src(DelayInp.PREV_ALU_OUT, 0)
        # blk7: S += u (scan); c1 <- D
        b[7].enable_alu(AluOp.ADD, AluInp.CURR_ALU_OUT, PD[0])
        b[7].enable_delay_from_src(DelayInp.PREV_ALU_OUT, 1)
        u.enable_output(OutSel.ALU_OUT, OutPath.WR0_LO)   # S
        u.enable_output(OutSel.DELAY_1, OutPath.WR0_HI)   # D
        return u

    def mk_seed():
        # Runs once (COUNT, repeat 1), consumes nothing, writes nothing;
        # zero-seeds the two scan flops (blk6=D, blk7=S) via the ZERO lane.
        u = UopConfig()
        for lane, src in ((1, InpSel.SRC_0), (2, InpSel.SRC_1),
                          (3, InpSel.SRC_0_HI), (4, InpSel.SRC_1_HI),
                          (5, InpSel.ZERO)):
            u.enable_input(src, lane)
        u.repeat_count = 1
        u.trigger = (Trigger.COUNT, Trigger.NONE, Trigger.NONE)
        u.next_uop = (1, 0, 0)
        b = u.datapath_config
        for k in range(8):
            b[k].enable_alu(AluOp.BYPASS, PD[4], PD[4])
            b[k].pass_through_delay(4)
        return u

    return [mk_seed(), mk_steady()]


def register_stats_op():
    """Register the fused dot/ssq prefix-scan op via the documented
    custom-DVE extension registry (concourse.dve_ops.OPS), with a
    hand-authored 2x_1p perf-mode program at table slot +1."""
    import concourse.dve_ops as dve_ops
    from concourse.dve_spec import (
        AluOp, One, Spec, Src0, Src1, Zero, lower, scan, select, sq,
    )
    from concourse.dve_uop import DveOpSpec
    from dataclasses import dataclass, field

    if _STATS_OP_NAME in dve_ops._SUB_OPCODE_FOR_NAME:
        for op in dve_ops.OPS:
            if op.name == _STATS_OP_NAME:
                return op

    rdot = scan(AluOp.ADD, Src0 * Src1)
    rssq = scan(AluOp.ADD, sq(Src0))
    par = scan(AluOp.LOGICAL_XOR, One, init=Zero)  # 1,0,1,0,... (k=0 -> 1)

    def _ref(in0, in1, c0, c1, c2):
        dot = np.cumsum((in0 * in1).astype(np.float32), axis=-1)
        ssq = np.cumsum((in0 * in0).astype(np.float32), axis=-1)
        k = np.arange(in0.shape[-1])
        return np.where(k % 2 == 1, dot, ssq).astype(np.float32)

    spec = Spec(body=select(par, rssq, rdot), reference=_ref)
    row = dve_ops._CUSTOM_DVE_ROW_BASE + len(dve_ops.OPS)
    assert row < 0x20
    shas = {}
    for ver in ("v3", "v4"):
        s = DveOpSpec(name=_STATS_OP_NAME, opcode=row,
                      uops=lower(spec, ver=ver), rd1_en=True)
        shas[ver] = s.sha(ver)

    uops_2x = _build_stats_uops_2x()
    for u in uops_2x:
        u.validate("v3")

    @dataclass(frozen=True)
    class DveOpPerf(dve_ops.DveOp):
        """DveOp whose v3 table also carries a 2x_1p program."""

        perf_uops: tuple = ()

        def compile(self, ver):
            base = super().compile(ver)
            if ver != "v3" or not self.perf_uops:
                return base
            return DveOpSpec(
                name=base.name,
                opcode=base.opcode,
                uops=base.uops,
                uops_2x=list(self.perf_uops),
                perf_max=STATS_PERF_MAX,
                rd1_en=base.rd1_en,
            )

    op = DveOpPerf(_STATS_OP_NAME, spec, subdim=False, uops_sha=shas,
                   perf_uops=tuple(uops_2x))
    dve_ops.OPS.append(op)
    dve_ops._SUB_OPCODE_FOR_NAME[_STATS_OP_NAME] = row
    dve_ops.CUSTOM_DVE_SPECS[_STATS_OP_NAME] = spec
    return op


def build_nc(
    rows_per_core=ROWS_PER_CORE,
    d=D,
    v_bufs=6,
    qs_bufs=4,
    copy_engs=("scalar", "scalar", "scalar", "scalar"),
    lhs_eng="vector",
    store_eng="gpsimd",
):
    n = N_STACK
    assert rows_per_core % 256 == 0
    ntiles = rows_per_core // 128
    stats_op = register_stats_op()
    nc = bacc.Bacc(
        "TRN2",
        target_bir_lowering=False,
        debug=False,
        enable_asserts=False,
    )
    VF = nc.dram_tensor("v", [rows_per_core, N_STACK, d], BF16,
                        kind="ExternalInput")
    QREP = nc.dram_tensor("qrep", [128, d], BF16, kind="ExternalInput")
    IDENT = nc.dram_tensor("ident", [128, 128], BF16, kind="ExternalInput")
    OUT = nc.dram_tensor("out", [rows_per_core, d], BF16, kind="ExternalOutput")

    mult = mybir.AluOpType.mult
    add = mybir.AluOpType.add
    AF = mybir.ActivationFunctionType
    AX = mybir.AxisListType

    with ExitStack() as ctx:
        tc = ctx.enter_context(tile.TileContext(nc))
        singles = ctx.enter_context(tc.tile_pool(name="singles", bufs=1))
        vpool = ctx.enter_context(tc.tile_pool(name="vpool", bufs=v_bufs))
        scpool = ctx.enter_context(tc.tile_pool(name="scpool", bufs=2))
        qspool = ctx.enter_context(tc.tile_pool(name="qspool", bufs=qs_bufs))
        outpool = ctx.enter_context(tc.tile_pool(name="outpool", bufs=2))
        lhspool = ctx.enter_context(tc.tile_pool(name="lhspool", bufs=2 * n))
        small = ctx.enter_context(tc.tile_pool(name="small", bufs=6))
        psum_o = ctx.enter_context(tc.tile_pool(name="psum_o", bufs=2, space="PSUM"))

        eng = {"vector": nc.vector, "gpsimd": nc.gpsimd, "scalar": nc.scalar,
               "sync": nc.sync}

        # Pin the ACT table set that serves Copy+Ln+Exp so the table-load
        # inserter never thrashes between per-function sets (21.8us in v3).
        from concourse.hw_specs import get_activation_tables

        tabs = get_activation_tables(nc.m.arch)
        atl_id = list(tabs.keys()).index("natural_log_exp_and_others")
        nc.scalar.add_instruction(
            mybir.InstLoadActFuncSet(
                name=nc.get_next_instruction_name(),
                act_func_set_id=atl_id,
                ins=[],
                outs=[],
            )
        )

        q_t = singles.tile([128, d], BF16)
        id_t = singles.tile([128, 128], BF16)
        warm_t = singles.tile([128, 4], BF16)
        # tiny transfers to spin up the sync/gpsimd DMA paths before the
        # first V tile lands on them
        nc.sync.dma_start(out=warm_t[:, 0:2], in_=IDENT.ap()[:, 0:2])
        nc.gpsimd.dma_start(out=warm_t[:, 2:4], in_=IDENT.ap()[:, 2:4])
        nc.scalar.dma_start(out=q_t[:, :], in_=QREP.ap())
        nc.scalar.dma_start(out=id_t[:, :], in_=IDENT.ap())
        zero_t = singles.tile([128, 1], F32)
        nc.vector.memset(zero_t[:, :], 0.0)
        eps_t = singles.tile([128, 1], F32)
        nc.vector.memset(eps_t[:, :], EPS)

        def emit_load_stats(it, fine=False):
            R = it * 128
            v_t = vpool.tile([128, n, d], BF16, tag="v", name=f"v{it}")
            if fine:
                # fine-grained loads: 512KiB-granularity deps so each per-n
                # stats op starts as soon as its own chunk lands (pipeline
                # ramp-up for the first pair, tail-latency for the last tile)
                nc.sync.dma_start(out=v_t[:, 0, :], in_=VF.ap()[R : R + 128, 0, :])
                nc.gpsimd.dma_start(out=v_t[:, 2, :], in_=VF.ap()[R : R + 128, 2, :])
                nc.sync.dma_start(out=v_t[:, 1, :], in_=VF.ap()[R : R + 128, 1, :])
                nc.gpsimd.dma_start(out=v_t[:, 3, :], in_=VF.ap()[R : R + 128, 3, :])
            else:
                # row-split halves: every descriptor covers a full 16KiB row,
                # halving descriptor count vs the n-split (DMA overhead)
                nc.sync.dma_start(
                    out=v_t[0:64, :, :], in_=VF.ap()[R : R + 64, :, :]
                )
                nc.gpsimd.dma_start(
                    out=v_t[64:128, :, :], in_=VF.ap()[R + 64 : R + 128, :, :]
                )
            # Per-n fused scans in 2x_1p perf mode (2 elems/cycle): the full
            # cum stream lands in an fp16 scratch tile (packed 16-bit dst is
            # what makes the op 2x-eligible); the last pair of each segment
            # holds (ssq total, dot total). One tiny CAST pulls all 4 pairs
            # into the f32 qs tile.
            sc = scpool.tile([128, n, d], FP16, tag="sc", name=f"sc{it}")
            for sn in range(n):
                inst = nc.vector._custom_dve(
                    stats_op,
                    out=sc[:, sn, :],
                    in0=v_t[:, sn, :],
                    in1=q_t[:, 0:d],
                )
                inst.ins.perf_max = STATS_PERF_MAX
            qs = qspool.tile([128, n, 2], F32, tag="qs", name=f"qs{it}")
            nc.vector.tensor_copy(out=qs[:, :, :], in_=sc[:, :, d - 2 : d])
            return v_t, qs

        def emit_smalls(ip, qs_a, qs_b):
            m = 2 * n
            # inv_rms = exp(-0.5 * ln(ssq/D + eps)) -- the /D+eps folds into
            # Ln's scale/bias, saving a Copy hop in the critical chain
            lnm = small.tile([128, m], F32, tag="lnm", name=f"lnm{ip}")
            for h, qs in enumerate((qs_a, qs_b)):
                nc.scalar.activation(
                    out=lnm[:, h * n : h * n + n],
                    in_=qs[:, :, 0],
                    func=AF.Ln,
                    bias=eps_t[:, :],
                    scale=1.0 / d,
                )
            inv = small.tile([128, m], F32, tag="inv", name=f"inv{ip}")
            nc.scalar.activation(
                out=inv[:, :], in_=lnm[:, :], func=AF.Exp,
                bias=zero_t[:, :], scale=-0.5,
            )
            # logits = dot * inv_rms
            lg = small.tile([128, m], F32, tag="lg", name=f"lg{ip}")
            for h, qs in enumerate((qs_a, qs_b)):
                nc.vector.tensor_tensor(
                    out=lg[:, h * n : h * n + n],
                    in0=qs[:, :, 1],
                    in1=inv[:, h * n : h * n + n],
                    op=mult,
                )
            e_t = small.tile([128, m], F32, tag="e", name=f"e{ip}")
            nc.scalar.activation(
                out=e_t[:, :], in_=lg[:, :], func=AF.Exp,
                bias=zero_t[:, :], scale=1.0,
            )
            s_t = small.tile([128, 2], F32, tag="s", name=f"s{ip}")
            e3 = e_t[:, :].rearrange("p (t j) -> p t j", j=n)
            nc.vector.tensor_reduce(
                out=s_t[:, :], in_=e3, axis=AX.X, op=add
            )
            si_t = small.tile([128, 2], F32, tag="si", name=f"si{ip}")
            nc.vector.reciprocal(out=si_t[:, :], in_=s_t[:, :])
            return e_t, si_t

        def emit_combine(it, v_t, e_t, si_t, h, cengs=None, sengs=None):
            R = it * 128
            cengs = cengs or copy_engs
            sengs = sengs or (store_eng, store_eng)
            # lhs = diag(e_n); the softmax denominator 1/s folds into the
            # PSUM->SBUF copy's per-partition scale instead
            lhs_list = []
            for sn in range(n):
                lhsT = lhspool.tile([128, 128], BF16, tag="lhs",
                                    name=f"lhs{it}_{sn}")
                if lhs_eng == "scalar":
                    nc.scalar.activation(
                        out=lhsT[:, :], in_=id_t[:, :], func=AF.Copy,
                        bias=0.0,
                        scale=e_t[:, h * n + sn : h * n + sn + 1],
                    )
                else:
                    eng[lhs_eng].tensor_scalar(
                        out=lhsT[:, :], in0=id_t[:, :],
                        scalar1=e_t[:, h * n + sn : h * n + sn + 1],
                        scalar2=None, op0=mult,
                    )
                lhs_list.append(lhsT)
            ps = psum_o.tile([128, d], F32, tag="ps", name=f"ps{it}")
            nchunk = d // 512
            for c in range(nchunk):
                for sn in range(n):
                    nc.tensor.matmul(
                        ps[:, c * 512 : (c + 1) * 512],
                        lhs_list[sn][:, :],
                        v_t[:, sn, c * 512 : (c + 1) * 512],
                        start=(sn == 0),
                        stop=(sn == n - 1),
                    )
            out_sb = outpool.tile([128, d], BF16, tag="osb", name=f"osb{it}")
            sislice = si_t[:, h : h + 1]
            for c in range(nchunk):
                ce = eng[cengs[c % len(cengs)]]
                if ce is nc.scalar:
                    nc.scalar.activation(
                        out=out_sb[:, c * 512 : (c + 1) * 512],
                        in_=ps[:, c * 512 : (c + 1) * 512],
                        func=AF.Copy, bias=0.0, scale=sislice,
                    )
                else:
                    ce.tensor_scalar(
                        out=out_sb[:, c * 512 : (c + 1) * 512],
                        in0=ps[:, c * 512 : (c + 1) * 512],
                        scalar1=sislice, scalar2=None, op0=mult,
                    )
            # one store per tile: full 4KiB-row descriptors
            eng[sengs[0]].dma_start(
                out=OUT.ap()[R : R + 128, :], in_=out_sb[:, :]
            )

        def emit_smalls_single(it, qs):
            m = n
            lnm = small.tile([128, m], F32, tag="lnm", name=f"lnm_s{it}")
            nc.scalar.activation(
                out=lnm[:, :], in_=qs[:, :, 0], func=AF.Ln,
                bias=eps_t[:, :], scale=1.0 / d,
            )
            inv = small.tile([128, m], F32, tag="inv", name=f"inv_s{it}")
            nc.scalar.activation(
                out=inv[:, :], in_=lnm[:, :], func=AF.Exp,
                bias=zero_t[:, :], scale=-0.5,
            )
            lg = small.tile([128, m], F32, tag="lg", name=f"lg_s{it}")
            nc.vector.tensor_tensor(
                out=lg[:, :], in0=qs[:, :, 1], in1=inv[:, :], op=mult,
            )
            e_t = small.tile([128, m], F32, tag="e", name=f"e_s{it}")
            nc.scalar.activation(
                out=e_t[:, :], in_=lg[:, :], func=AF.Exp,
                bias=zero_t[:, :], scale=1.0,
            )
            s_t = small.tile([128, 1], F32, tag="s", name=f"s_s{it}")
            nc.vector.tensor_reduce(
                out=s_t[:, :], in_=e_t[:, :], axis=AX.X, op=add
            )
            si_t = small.tile([128, 1], F32, tag="si", name=f"si_s{it}")
            nc.vector.reciprocal(out=si_t[:, :], in_=s_t[:, :])
            return e_t, si_t

        assert ntiles % 2 == 0
        # pairs for tiles 0..ntiles-3; the last two tiles run tile-granular
        # so combine(n-2) overlaps stats(n-1) and the tail is one tile deep.
        pending = None
        for p in range(ntiles // 2 - 1):
            v_a, qs_a = emit_load_stats(2 * p, fine=(p == 0))
            v_b, qs_b = emit_load_stats(2 * p + 1, fine=(p == 0))
            cur = (p, v_a, v_b, qs_a, qs_b)
            if pending is not None:
                pp, va, vb, qa, qb = pending
                e_p, si_p = emit_smalls(pp, qa, qb)
                emit_combine(2 * pp, va, e_p, si_p, 0)
                emit_combine(2 * pp + 1, vb, e_p, si_p, 1)
            pending = cur
        # last two tiles, stats emitted tile by tile
        t0i = ntiles - 2
        v_x, qs_x = emit_load_stats(t0i)
        # flush the pending pair while tile t0i's stats run
        if pending is not None:
            pp, va, vb, qa, qb = pending
            e_p, si_p = emit_smalls(pp, qa, qb)
            emit_combine(2 * pp, va, e_p, si_p, 0)
            emit_combine(2 * pp + 1, vb, e_p, si_p, 1)
        e_x, si_x = emit_smalls_single(t0i, qs_x)
        v_y, qs_y = emit_load_stats(t0i + 1, fine=True)
        emit_combine(t0i, v_x, e_x, si_x, 0, cengs=("vector",))
        e_y, si_y = emit_smalls_single(t0i + 1, qs_y)
        emit_combine(t0i + 1, v_y, e_y, si_y, 0, cengs=("vector",),
                     sengs=("sync",))

    nc.compile()
    return nc


def make_in_maps(V_flat, query, rows_per_core, n_cores):
    import ml_dtypes

    d = V_flat.shape[2]
    qrep = np.ascontiguousarray(
        np.broadcast_to(query.astype(ml_dtypes.bfloat16)[None, :], (128, d))
    )
    ident = np.eye(128, dtype=ml_dtypes.bfloat16)
    Vb16 = V_flat.astype(ml_dtypes.bfloat16)
    in_maps = []
    for c in range(n_cores):
        sl = slice(c * rows_per_core, (c + 1) * rows_per_core)
        v = np.ascontiguousarray(Vb16[:, sl, :].transpose(1, 0, 2))
        in_maps.append({"v": v, "qrep": qrep, "ident": ident})
    return in_maps


_CACHE = {}


def _get_nc():
    if "nc" not in _CACHE:
        _CACHE["nc"] = build_nc()
    return _CACHE["nc"]


def kernel(V, query):
    V = np.asarray(V, dtype=np.float32)
    query = np.asarray(query, dtype=np.float32)
    assert V.shape == (N_STACK, B, T, D)
    nc = _get_nc()
    V_flat = V.reshape(N_STACK, ROWS, D)
    in_maps = make_in_maps(V_flat, query, ROWS_PER_CORE, N_CORES)
    res = run_bass_kernel_spmd(nc, in_maps, core_ids=list(range(N_CORES)))
    out = np.concatenate(
        [np.asarray(res.results[c]["out"]).astype(np.float32)
         for c in range(N_CORES)],
        axis=0,
    )
    return out.reshape(B, T, D)


if __name__ == "__main__":
    rng = np.random.default_rng(0)
    V = rng.standard_normal((N_STACK, B, T, D), dtype=np.float32)
    q = (rng.standard_normal(D) * 0.01).astype(np.float32)
    out = kernel(V, q)
    print("out", out.shape, out.dtype, float(np.abs(out).mean()))



# revision 23
# speedup vs baseline: 1.2289x; 1.2289x over previous
"""Trainium2 Bass kernel for nn_AttnResModule (pooling / memory-bound).

Computation (reference):
    inv_rms = rsqrt(mean(V*V, -1) + eps)
    logits  = einsum('d,nbtd->nbt', query, V) * inv_rms
    w       = softmax(logits, axis=0)            # over stack axis n=4
    out     = einsum('nbt,nbtd->btd', w, V)

Design (one row per partition; tile = [128 rows, (n=4, d=2048)] bf16):
  - stats: ONE fused custom-DVE scan op per (n, tile) computes BOTH
    q.V and sum(V^2) in a single 1-elem/cycle pass, interleaved by a
    parity scan:  out[k] = k odd ? cumsum(V*q)[k] : cumsum(V*V)[k].
    The dst AP is a stride-0 broadcast, so all 2048 writes land on 2
    columns and the final values are (ssq at col 0, dot at col 1).
    64 ops x ~2.28us = 146us on DVE -- the kernel's critical engine.
    (Alternatives measured on HW: STT+accum 2287ns *per stat*, ACT
    Square+accum 2320ns; tensor_reduce has no perf mode, 2750ns.)
  - inv_rms = exp(-0.5*ln(ssq/D + eps)) on ACT; /D+eps folded into
    Ln's scale/bias. The ACT table set (natural_log_exp_and_others:
    Ln+Exp+Copy) is pinned with an explicit InstLoadActFuncSet, which
    kills the inserter's 29-reload thrash (37us in an earlier rev).
  - softmax: Exp on ACT; sum over n is an in-partition tiny reduce;
    the 1/sum normalization folds into the PSUM->SBUF copies' scale,
    so combine weights are just diag(e_n).
  - combine: PE matmuls with diagonal lhsT built by ACT Copy with a
    per-partition AP scale; PSUM accumulates over n; chunk-major order
    so copies start early.
  - output: PSUM f32 -> SBUF bf16 copy-with-scale on ACT (vector for
    the last two tiles, ACT queue latency gates the tail), split
    stores via gpsimd SWDGE; host upcasts to f32 (store traffic halved)
  - pipeline: pair-delayed; the last two tiles run tile-granular so the
    second-to-last combine overlaps the last stats. v_bufs=6 is a
    measured optimum: deeper DMA look-ahead slows every engine via
    SBUF port contention (custom op 2284 -> 2740ns at v_bufs=10).

Sharding: data-parallel over rows; 8 cores x 2048 rows, no
communication. Measured: 257.7us (v1 baseline) -> ~180us, rel err 3.1e-3.
"""

import sys
from contextlib import ExitStack

import numpy as np

_TRN_REPO = "/opt/trn_rl_repo"
if _TRN_REPO not in sys.path:
    sys.path.insert(0, _TRN_REPO)

import concourse.bacc as bacc
import concourse.tile as tile
from concourse import mybir
from concourse.bass_utils import run_bass_kernel_spmd

N_STACK = 4
B = 4
T = 4096
D = 2048
N_CORES = 8
ROWS = B * T
ROWS_PER_CORE = ROWS // N_CORES
EPS = float(np.finfo(np.float32).eps)
F32 = mybir.dt.float32
BF16 = mybir.dt.bfloat16
FP16 = mybir.dt.float16

_STATS_OP_NAME = "ATTN_STATS_ANT"
STATS_PERF_MAX = 1  # byte-36[7:6]: highest engine-reachable perf slot (2x_1p)


def _build_stats_uops_2x():
    """Hand-authored 2x_1p datapath program for the fused dot/ssq op.

    Per cycle the engine delivers a packed pair: e0=SRC_0, e1=SRC_0_HI,
    q0=SRC_1, q1=SRC_1_HI, and writes the packed pair WR0_LO/WR0_HI.

        blk0: s0 = e0*e0          blk4: t = m0+m1
        blk1: m0 = e0*q0          blk5: u = s0+s1
        blk2: m1 = e1*q1          blk6: D += t   (CURR feedback scan)
        blk3: s1 = e1*e1          blk7: S += u   (CURR feedback scan)
        out:  WR0_LO = S (ssq cum incl. both), WR0_HI = D (dot cum)

    Mid-stream even slots differ from the 1x parity-select program (they
    include the pair's odd square), but consumers only read the last
    pair, where S/D are the full-stream ssq/dot.
    """
    from concourse.dve_uop import (
        AluInp, AluOp, DelayInp, InpSel, OutPath, OutSel, Trigger, UopConfig,
    )

    PD = [AluInp.PREV_DELAY_0, AluInp.PREV_DELAY_1, AluInp.PREV_DELAY_2,
          AluInp.PREV_DELAY_3, AluInp.PREV_DELAY_4]

    def mk_steady():
        u = UopConfig()
        for lane, src in ((1, InpSel.SRC_0), (2, InpSel.SRC_1),
                          (3, InpSel.SRC_0_HI), (4, InpSel.SRC_1_HI),
                          (5, InpSel.ZERO)):
            u.enable_input(src, lane)
        u.require_inp0 = 1
        u.require_inp1 = 1
        u.trigger = (Trigger.SRC_TENSOR_DONE, Trigger.NONE, Trigger.NONE)
        u.next_uop = (0, 0, 0)
        b = u.datapath_config
        # blk0: s0 = e0*e0; carry e0,q0,e1,q1
        b[0].enable_alu(AluOp.MULTIPLY, PD[0], PD[0])
        b[0].pass_through_delay(0, 1, 2, 3)
        # blk1: m0 = e0*q0; c0 <- s0, carry e1,q1
        b[1].enable_alu(AluOp.MULTIPLY, PD[0], PD[1])
        b[1].enable_delay_from_src(DelayInp.PREV_ALU_OUT, 0)
        b[1].pass_through_delay(2, 3)
        # blk2: m1 = e1*q1; c1 <- m0, carry s0,e1
        b[2].enable_alu(AluOp.MULTIPLY, PD[2], PD[3])
        b[2].enable_delay_from_src(DelayInp.PREV_ALU_OUT, 1)
        b[2].pass_through_delay(0, 2)
        # blk3: s1 = e1*e1; c3 <- m1, carry s0,m0
        b[3].enable_alu(AluOp.MULTIPLY, PD[2], PD[2])
        b[3].enable_delay_from_src(DelayInp.PREV_ALU_OUT, 3)
        b[3].pass_through_delay(0, 1)
        # blk4: t = m0+m1; c2 <- s1, carry s0
        b[4].enable_alu(AluOp.ADD, PD[1], PD[3])
        b[4].enable_delay_from_src(DelayInp.PREV_ALU_OUT, 2)
        b[4].pass_through_delay(0)
        # blk5: u = s0+s1; c1 <- t
        b[5].enable_alu(AluOp.ADD, PD[0], PD[2])
        b[5].enable_delay_from_src(DelayInp.PREV_ALU_OUT, 1)
        # blk6: D += t (scan); c0 <- u
        b[6].enable_alu(AluOp.ADD, AluInp.CURR_ALU_OUT, PD[1])
        b[6].enable_delay_from_src(DelayInp.PREV_ALU_OUT, 0)
        # blk7: S += u (scan); c1 <- D
        b[7].enable_alu(AluOp.ADD, AluInp.CURR_ALU_OUT, PD[0])
        b[7].enable_delay_from_src(DelayInp.PREV_ALU_OUT, 1)
        u.enable_output(OutSel.ALU_OUT, OutPath.WR0_LO)   # S
        u.enable_output(OutSel.DELAY_1, OutPath.WR0_HI)   # D
        return u

    def mk_seed():
        # Runs once (COUNT, repeat 1), consumes nothing, writes nothing;
        # zero-seeds the two scan flops (blk6=D, blk7=S) via the ZERO lane.
        u = UopConfig()
        for lane, src in ((1, InpSel.SRC_0), (2, InpSel.SRC_1),
                          (3, InpSel.SRC_0_HI), (4, InpSel.SRC_1_HI),
                          (5, InpSel.ZERO)):
            u.enable_input(src, lane)
        u.repeat_count = 1
        u.trigger = (Trigger.COUNT, Trigger.NONE, Trigger.NONE)
        u.next_uop = (1, 0, 0)
        b = u.datapath_config
        for k in range(8):
            b[k].enable_alu(AluOp.BYPASS, PD[4], PD[4])
            b[k].pass_through_delay(4)
        return u

    return [mk_seed(), mk_steady()]


def register_stats_op():
    """Register the fused dot/ssq prefix-scan op via the documented
    custom-DVE extension registry (concourse.dve_ops.OPS), with a
    hand-authored 2x_1p perf-mode program at table slot +1."""
    import concourse.dve_ops as dve_ops
    from concourse.dve_spec import (
        AluOp, One, Spec, Src0, Src1, Zero, lower, scan, select, sq,
    )
    from concourse.dve_uop import DveOpSpec
    from dataclasses import dataclass, field

    if _STATS_OP_NAME in dve_ops._SUB_OPCODE_FOR_NAME:
        for op in dve_ops.OPS:
            if op.name == _STATS_OP_NAME:
                return op

    rdot = scan(AluOp.ADD, Src0 * Src1)
    rssq = scan(AluOp.ADD, sq(Src0))
    par = scan(AluOp.LOGICAL_XOR, One, init=Zero)  # 1,0,1,0,... (k=0 -> 1)

    def _ref(in0, in1, c0, c1, c2):
        dot = np.cumsum((in0 * in1).astype(np.float32), axis=-1)
        ssq = np.cumsum((in0 * in0).astype(np.float32), axis=-1)
        k = np.arange(in0.shape[-1])
        return np.where(k % 2 == 1, dot, ssq).astype(np.float32)

    spec = Spec(body=select(par, rssq, rdot), reference=_ref)
    row = dve_ops._CUSTOM_DVE_ROW_BASE + len(dve_ops.OPS)
    assert row < 0x20
    shas = {}
    for ver in ("v3", "v4"):
        s = DveOpSpec(name=_STATS_OP_NAME, opcode=row,
                      uops=lower(spec, ver=ver), rd1_en=True)
        shas[ver] = s.sha(ver)

    uops_2x = _build_stats_uops_2x()
    for u in uops_2x:
        u.validate("v3")

    @dataclass(frozen=True)
    class DveOpPerf(dve_ops.DveOp):
        """DveOp whose v3 table also carries a 2x_1p program."""

        perf_uops: tuple = ()

        def compile(self, ver):
            base = super().compile(ver)
            if ver != "v3" or not self.perf_uops:
                return base
            return DveOpSpec(
                name=base.name,
                opcode=base.opcode,
                uops=base.uops,
                uops_2x=list(self.perf_uops),
                perf_max=STATS_PERF_MAX,
                rd1_en=base.rd1_en,
            )

    op = DveOpPerf(_STATS_OP_NAME, spec, subdim=False, uops_sha=shas,
                   perf_uops=tuple(uops_2x))
    dve_ops.OPS.append(op)
    dve_ops._SUB_OPCODE_FOR_NAME[_STATS_OP_NAME] = row
    dve_ops.CUSTOM_DVE_SPECS[_STATS_OP_NAME] = spec
    return op


def build_nc(
    rows_per_core=ROWS_PER_CORE,
    d=D,
    v_bufs=6,
    qs_bufs=4,
    copy_engs=("scalar", "scalar", "scalar", "scalar"),
    lhs_eng="vector",
    store_eng="gpsimd",
):
    n = N_STACK
    assert rows_per_core % 256 == 0
    ntiles = rows_per_core // 128
    stats_op = register_stats_op()
    nc = bacc.Bacc(
        "TRN2",
        target_bir_lowering=False,
        debug=False,
        enable_asserts=False,
    )
    VF = nc.dram_tensor("v", [rows_per_core, N_STACK, d], BF16,
                        kind="ExternalInput")
    QREP = nc.dram_tensor("qrep", [128, d], BF16, kind="ExternalInput")
    IDENT = nc.dram_tensor("ident", [128, 128], BF16, kind="ExternalInput")
    OUT = nc.dram_tensor("out", [rows_per_core, d], BF16, kind="ExternalOutput")

    mult = mybir.AluOpType.mult
    add = mybir.AluOpType.add
    AF = mybir.ActivationFunctionType
    AX = mybir.AxisListType

    with ExitStack() as ctx:
        tc = ctx.enter_context(tile.TileContext(nc))
        singles = ctx.enter_context(tc.tile_pool(name="singles", bufs=1))
        vpool = ctx.enter_context(tc.tile_pool(name="vpool", bufs=v_bufs))
        scpool = ctx.enter_context(tc.tile_pool(name="scpool", bufs=2))
        qspool = ctx.enter_context(tc.tile_pool(name="qspool", bufs=qs_bufs))
        outpool = ctx.enter_context(tc.tile_pool(name="outpool", bufs=2))
        lhspool = ctx.enter_context(tc.tile_pool(name="lhspool", bufs=2 * n))
        small = ctx.enter_context(tc.tile_pool(name="small", bufs=6))
        psum_o = ctx.enter_context(tc.tile_pool(name="psum_o", bufs=2, space="PSUM"))

        eng = {"vector": nc.vector, "gpsimd": nc.gpsimd, "scalar": nc.scalar,
               "sync": nc.sync}

        # Pin the ACT table set that serves Copy+Ln+Exp so the table-load
        # inserter never thrashes between per-function sets (21.8us in v3).
        from concourse.hw_specs import get_activation_tables

        tabs = get_activation_tables(nc.m.arch)
        atl_id = list(tabs.keys()).index("natural_log_exp_and_others")
        nc.scalar.add_instruction(
            mybir.InstLoadActFuncSet(
                name=nc.get_next_instruction_name(),
                act_func_set_id=atl_id,
                ins=[],
                outs=[],
            )
        )

        q_t = singles.tile([128, d], BF16)
        id_t = singles.tile([128, 128], BF16)
        warm_t = singles.tile([128, 4], BF16)
        # tiny transfers to spin up the sync/gpsimd DMA paths before the
        # first V tile lands on them
        nc.sync.dma_start(out=warm_t[:, 0:2], in_=IDENT.ap()[:, 0:2])
        nc.gpsimd.dma_start(out=warm_t[:, 2:4], in_=IDENT.ap()[:, 2:4])
        nc.scalar.dma_start(out=q_t[:, :], in_=QREP.ap())
        nc.scalar.dma_start(out=id_t[:, :], in_=IDENT.ap())
        zero_t = singles.tile([128, 1], F32)
        nc.vector.memset(zero_t[:, :], 0.0)
        eps_t = singles.tile([128, 1], F32)
        nc.vector.memset(eps_t[:, :], EPS)

        def emit_load_stats(it, fine=False):
            R = it * 128
            v_t = vpool.tile([128, n, d], BF16, tag="v", name=f"v{it}")
            if fine:
                # fine-grained loads: 512KiB-granularity deps so each per-n
                # stats op starts as soon as its own chunk lands (pipeline
                # ramp-up for the first pair, tail-latency for the last tile)
                nc.sync.dma_start(out=v_t[:, 0, :], in_=VF.ap()[R : R + 128, 0, :])
                nc.gpsimd.dma_start(out=v_t[:, 2, :], in_=VF.ap()[R : R + 128, 2, :])
                nc.sync.dma_start(out=v_t[:, 1, :], in_=VF.ap()[R : R + 128, 1, :])
                nc.gpsimd.dma_start(out=v_t[:, 3, :], in_=VF.ap()[R : R + 128, 3, :])
            else:
                nc.sync.dma_start(
                    out=v_t[:, 0:2, :], in_=VF.ap()[R : R + 128, 0:2, :]
                )
                nc.gpsimd.dma_start(
                    out=v_t[:, 2:4, :], in_=VF.ap()[R : R + 128, 2:4, :]
                )
            # Per-n fused scans in 2x_1p perf mode (2 elems/cycle): the full
            # cum stream lands in an fp16 scratch tile (packed 16-bit dst is
            # what makes the op 2x-eligible); the last pair of each segment
            # holds (ssq total, dot total). One tiny CAST pulls all 4 pairs
            # into the f32 qs tile.
            sc = scpool.tile([128, n, d], FP16, tag="sc", name=f"sc{it}")
            for sn in range(n):
                inst = nc.vector._custom_dve(
                    stats_op,
                    out=sc[:, sn, :],
                    in0=v_t[:, sn, :],
                    in1=q_t[:, 0:d],
                )
                inst.ins.perf_max = STATS_PERF_MAX
            qs = qspool.tile([128, n, 2], F32, tag="qs", name=f"qs{it}")
            nc.vector.tensor_copy(out=qs[:, :, :], in_=sc[:, :, d - 2 : d])
            return v_t, qs

        def emit_smalls(ip, qs_a, qs_b):
            m = 2 * n
            # inv_rms = exp(-0.5 * ln(ssq/D + eps)) -- the /D+eps folds into
            # Ln's scale/bias, saving a Copy hop in the critical chain
            lnm = small.tile([128, m], F32, tag="lnm", name=f"lnm{ip}")
            for h, qs in enumerate((qs_a, qs_b)):
                nc.scalar.activation(
                    out=lnm[:, h * n : h * n + n],
                    in_=qs[:, :, 0],
                    func=AF.Ln,
                    bias=eps_t[:, :],
                    scale=1.0 / d,
                )
            inv = small.tile([128, m], F32, tag="inv", name=f"inv{ip}")
            nc.scalar.activation(
                out=inv[:, :], in_=lnm[:, :], func=AF.Exp,
                bias=zero_t[:, :], scale=-0.5,
            )
            # logits = dot * inv_rms
            lg = small.tile([128, m], F32, tag="lg", name=f"lg{ip}")
            for h, qs in enumerate((qs_a, qs_b)):
                nc.vector.tensor_tensor(
                    out=lg[:, h * n : h * n + n],
                    in0=qs[:, :, 1],
                    in1=inv[:, h * n : h * n + n],
                    op=mult,
                )
            e_t = small.tile([128, m], F32, tag="e", name=f"e{ip}")
            nc.scalar.activation(
                out=e_t[:, :], in_=lg[:, :], func=AF.Exp,
                bias=zero_t[:, :], scale=1.0,
            )
            s_t = small.tile([128, 2], F32, tag="s", name=f"s{ip}")
            e3 = e_t[:, :].rearrange("p (t j) -> p t j", j=n)
            nc.vector.tensor_reduce(
                out=s_t[:, :], in_=e3, axis=AX.X, op=add
            )
            si_t = small.tile([128, 2], F32, tag="si", name=f"si{ip}")
            nc.vector.reciprocal(out=si_t[:, :], in_=s_t[:, :])
            return e_t, si_t

        def emit_combine(it, v_t, e_t, si_t, h, cengs=None, sengs=None):
            R = it * 128
            cengs = cengs or copy_engs
            sengs = sengs or (store_eng, store_eng)
            # lhs = diag(e_n); the softmax denominator 1/s folds into the
            # PSUM->SBUF copy's per-partition scale instead
            lhs_list = []
            for sn in range(n):
                lhsT = lhspool.tile([128, 128], BF16, tag="lhs",
                                    name=f"lhs{it}_{sn}")
                if lhs_eng == "scalar":
                    nc.scalar.activation(
                        out=lhsT[:, :], in_=id_t[:, :], func=AF.Copy,
                        bias=0.0,
                        scale=e_t[:, h * n + sn : h * n + sn + 1],
                    )
                else:
                    eng[lhs_eng].tensor_scalar(
                        out=lhsT[:, :], in0=id_t[:, :],
                        scalar1=e_t[:, h * n + sn : h * n + sn + 1],
                        scalar2=None, op0=mult,
                    )
                lhs_list.append(lhsT)
            ps = psum_o.tile([128, d], F32, tag="ps", name=f"ps{it}")
            nchunk = d // 512
            for c in range(nchunk):
                for sn in range(n):
                    nc.tensor.matmul(
                        ps[:, c * 512 : (c + 1) * 512],
                        lhs_list[sn][:, :],
                        v_t[:, sn, c * 512 : (c + 1) * 512],
                        start=(sn == 0),
                        stop=(sn == n - 1),
                    )
            out_sb = outpool.tile([128, d], BF16, tag="osb", name=f"osb{it}")
            sislice = si_t[:, h : h + 1]
            for c in range(nchunk):
                ce = eng[cengs[c % len(cengs)]]
                if ce is nc.scalar:
                    nc.scalar.activation(
                        out=out_sb[:, c * 512 : (c + 1) * 512],
                        in_=ps[:, c * 512 : (c + 1) * 512],
                        func=AF.Copy, bias=0.0, scale=sislice,
                    )
                else:
                    ce.tensor_scalar(
                        out=out_sb[:, c * 512 : (c + 1) * 512],
                        in0=ps[:, c * 512 : (c + 1) * 512],
                        scalar1=sislice, scalar2=None, op0=mult,
                    )
            for hh in range(2):
                hw = d // 2
                eng[sengs[hh % len(sengs)]].dma_start(
                    out=OUT.ap()[R : R + 128, hh * hw : (hh + 1) * hw],
                    in_=out_sb[:, hh * hw : (hh + 1) * hw],
                )

        def emit_smalls_single(it, qs):
            m = n
            lnm = small.tile([128, m], F32, tag="lnm", name=f"lnm_s{it}")
            nc.scalar.activation(
                out=lnm[:, :], in_=qs[:, :, 0], func=AF.Ln,
                bias=eps_t[:, :], scale=1.0 / d,
            )
            inv = small.tile([128, m], F32, tag="inv", name=f"inv_s{it}")
            nc.scalar.activation(
                out=inv[:, :], in_=lnm[:, :], func=AF.Exp,
                bias=zero_t[:, :], scale=-0.5,
            )
            lg = small.tile([128, m], F32, tag="lg", name=f"lg_s{it}")
            nc.vector.tensor_tensor(
                out=lg[:, :], in0=qs[:, :, 1], in1=inv[:, :], op=mult,
            )
            e_t = small.tile([128, m], F32, tag="e", name=f"e_s{it}")
            nc.scalar.activation(
                out=e_t[:, :], in_=lg[:, :], func=AF.Exp,
                bias=zero_t[:, :], scale=1.0,
            )
            s_t = small.tile([128, 1], F32, tag="s", name=f"s_s{it}")
            nc.vector.tensor_reduce(
                out=s_t[:, :], in_=e_t[:, :], axis=AX.X, op=add
            )
            si_t = small.tile([128, 1], F32, tag="si", name=f"si_s{it}")
            nc.vector.reciprocal(out=si_t[:, :], in_=s_t[:, :])
            return e_t, si_t

        assert ntiles % 2 == 0
        # pairs for tiles 0..ntiles-3; the last two tiles run tile-granular
        # so combine(n-2) overlaps stats(n-1) and the tail is one tile deep.
        pending = None
        for p in range(ntiles // 2 - 1):
            v_a, qs_a = emit_load_stats(2 * p, fine=(p == 0))
            v_b, qs_b = emit_load_stats(2 * p + 1, fine=(p == 0))
            cur = (p, v_a, v_b, qs_a, qs_b)
            if pending is not None:
                pp, va, vb, qa, qb = pending
                e_p, si_p = emit_smalls(pp, qa, qb)
                emit_combine(2 * pp, va, e_p, si_p, 0)
                emit_combine(2 * pp + 1, vb, e_p, si_p, 1)
            pending = cur
        # last two tiles, stats emitted tile by tile
        t0i = ntiles - 2
        v_x, qs_x = emit_load_stats(t0i)
        # flush the pending pair while tile t0i's stats run
        if pending is not None:
            pp, va, vb, qa, qb = pending
            e_p, si_p = emit_smalls(pp, qa, qb)
            emit_combine(2 * pp, va, e_p, si_p, 0)
            emit_combine(2 * pp + 1, vb, e_p, si_p, 1)
        e_x, si_x = emit_smalls_single(t0i, qs_x)
        v_y, qs_y = emit_load_stats(t0i + 1, fine=True)
        emit_combine(t0i, v_x, e_x, si_x, 0, cengs=("vector",))
        e_y, si_y = emit_smalls_single(t0i + 1, qs_y)
        emit_combine(t0i + 1, v_y, e_y, si_y, 0, cengs=("vector",),
                     sengs=("sync", "scalar"))

    nc.compile()
    return nc


def make_in_maps(V_flat, query, rows_per_core, n_cores):
    import ml_dtypes

    d = V_flat.shape[2]
    qrep = np.ascontiguousarray(
        np.broadcast_to(query.astype(ml_dtypes.bfloat16)[None, :], (128, d))
    )
    ident = np.eye(128, dtype=ml_dtypes.bfloat16)
    Vb16 = V_flat.astype(ml_dtypes.bfloat16)
    in_maps = []
    for c in range(n_cores):
        sl = slice(c * rows_per_core, (c + 1) * rows_per_core)
        v = np.ascontiguousarray(Vb16[:, sl, :].transpose(1, 0, 2))
        in_maps.append({"v": v, "qrep": qrep, "ident": ident})
    return in_maps


_CACHE = {}


def _get_nc():
    if "nc" not in _CACHE:
        _CACHE["nc"] = build_nc()
    return _CACHE["nc"]


def kernel(V, query):
    V = np.asarray(V, dtype=np.float32)
    query = np.asarray(query, dtype=np.float32)
    assert V.shape == (N_STACK, B, T, D)
    nc = _get_nc()
    V_flat = V.reshape(N_STACK, ROWS, D)
    in_maps = make_in_maps(V_flat, query, ROWS_PER_CORE, N_CORES)
    res = run_bass_kernel_spmd(nc, in_maps, core_ids=list(range(N_CORES)))
    out = np.concatenate(
        [np.asarray(res.results[c]["out"]).astype(np.float32)
         for c in range(N_CORES)],
        axis=0,
    )
    return out.reshape(B, T, D)


if __name__ == "__main__":
    rng = np.random.default_rng(0)
    V = rng.standard_normal((N_STACK, B, T, D), dtype=np.float32)
    q = (rng.standard_normal(D) * 0.01).astype(np.float32)
    out = kernel(V, q)
    print("out", out.shape, out.dtype, float(np.abs(out).mean()))

